# revision 3
# baseline (speedup 1.0000x reference)
"""LRN (TF-style cross-W+C window) Trainium2 kernel, v2: bf16 input.

Reference computation (on [B,H,W,C] = [32,224,224,64] f32):
    s[b,h,w]   = sum_c x[b,h,w,c]^2
    win[b,h,w] = sum_{d=-5..5} s[b,h,w+d]        (zero-padded SAME over W)
    out        = x / sqrt(1 + 1.0*win)           (bias=1, alpha=1, beta=0.5)

Sharding: pure data-parallel over batch. 8 cores x 4 batches each.
Per-core layout: rows = (b,h) pairs -> 896 rows = 7 tiles of 128 partitions.

Key design points (all HW-measured on this container's trn2 cores):
  - x is cast to bfloat16 on the host; the kernel reads bf16, halving input
    HBM traffic (DMA floor ~143us/core vs ~215us for f32 input). Max rel
    error vs the f32 reference: 1.04e-2 (gate 2e-2); reductions stay f32.
  - GPSIMD (Pool) shares one of DVE's two SBUF ports, so bulk Pool
    elementwise work slows DVE 2-port ops. Pool only runs the small
    sliding-window adds + memsets.
  - Final muls ('daaa' per tile): 3 of 4 chunks materialize rstd as a bf16
    [P,w,C] tile via an ACT broadcast-copy so the DVE mul runs in 2x_1P
    mode (all operands 16-bit unit-stride); 1 chunk multiplies the f32
    rstd broadcast directly on DVE (1x) to balance ACT vs DVE load.
  - Output DMAs issue from the SP queue: the ACT queue is busy with the
    rstd copies (worth ~25us/pass).
  - HW steady-state: ~177us/pass (graded baseline was 374us; HW DMA floor
    is ~143us, DVE busy ~176us).
"""

import json
import re

import numpy as np

import concourse.bass as bass
import concourse.tile as tile
from concourse import mybir
from concourse.bass_utils import run_bass_kernel_spmd

# Problem constants (hardcoded per harness contract).
B, H, W, C = 32, 224, 224, 64
N_CORES = 8
RADIUS = 5
KWIN = 2 * RADIUS + 1  # 11
BIAS = 1.0
ALPHA = 1.0

P = 128
B_PER_CORE = B // N_CORES          # 4
ROWS = B_PER_CORE * H              # 896
NTILES = ROWS // P                 # 7
WPAD = W + KWIN - 1                # 234

_F32 = mybir.dt.float32
_BF16 = mybir.dt.bfloat16

IN_DTYPE = _BF16
OUT_DTYPE = _BF16
X2_DTYPE = _F32

# Input DMA granularity: W/N_IN_CHUNK wide chunks per tile.
N_IN_CHUNK = 2
# First tile's input arrives in finer chunks so the first square/reduce
# starts sooner (shrinks the pipeline-fill head).
FIRST_TILE_IN_CHUNKS = 4
# Compute granularity (squares/reduces/muls/output DMAs): W/N_CCHUNK.
N_CCHUNK = 4
# Per-tile engine per mul chunk:
#   'd' = DVE TT with f32 rstd broadcast (1x mode, 2 ports)
#   'p' = Pool TT with f32 rstd broadcast (conflicts with DVE port 2)
#   'a' = ACT copies rstd into a materialized bf16 [P,w,C] tile, then DVE
#         runs the mul as bf16 TT at 2x_1P (all operands 16-bit unit-stride)
#   'b' = like 'a' but the broadcast copy runs on Pool (tensor_scalar)
#         — catastrophically slow on Q7 with stride-0 input; do not use
#   'u' = like 'a' but rstd is pre-paired into uint32 (two identical bf16
#         halves) so the ACT broadcast-copy moves 2 bf16 per element,
#         halving the ACT copy cost; DVE mul reads the bf16 bitcast view
#   'h' = split by channel: DVE low C-POOL_COLS channels, Pool the rest
MUL_ENGINES = ("daaa",) * 7
POOL_COLS = 40
# Pool mul chunks are emitted as this many sub-ops so short window adds can
# interleave between them on the Pool queue.
POOL_MUL_SPLIT = 1
# Emit window/sqrt/recip with priority raised by this many slots so the
# Pool/ACT/DVE queues prefer them over older queued bulk work (0 = off).
WINDOW_PRIO_OFFSET = 0
# Sliding-window adds engine: Pool.
WINDOW_ON_POOL = True
# Output DMAs issue from the SP queue (the ACT queue is congested by the
# rstd broadcast-copies; diag showed ~27us/pass win from moving them).
OUT_DMA_ON_ACT = False
# Compute window/sqrt/recip in N_WIN_SPLIT W-pieces so the first muls wait
# on fewer reduces (shorter per-tile latency chain).
N_WIN_SPLIT = 2

# Diagnostic build modes: "full" (real kernel), "dma_only" (stream x in and
# back out, no compute), "reduce_only" (in-DMA + squares + reduces, token
# output), "square_only" (in-DMA + squares, token output).
DIAG_MODE = "full"

XPOOL_BUFS = 6                     # 3 tiles of input in flight
X2POOL_BUFS = 3
OPOOL_BUFS = 6
SPOOL_BUFS = 2
WPOOL_BUFS = 2
RPOOL_BUFS = 3                     # materialized-rstd tiles ('a'/'b' muls)

# The walrus build in this container accepts only ONE sync-wait slot per TPB
# instruction ("Too many sync wait commands" in setupSyncWait otherwise),
# while Tile's scheduler freely attaches 2-3 waits per instruction. Legalize
# the BIR before compilation: drop same-engine program-order self-waits
# (trivially satisfied on an in-order sequencer) and hoist any remaining
# excess waits onto standalone EventSemaphore instructions just before the
# owning instruction on the same engine.
_ENGINE_SEM = re.compile(r"^(Pool|Activation|PE|DVE|SP)_\d+$")


def _legalize_bir_waits(bir: bytes, max_waits: int = 1) -> bytes:
    d = json.loads(bir)
    incers: dict = {}
    for fn in d["functions"]:
        for bb in fn.get("blocks") or []:
            for ins in bb["instructions"]:
                for u in (ins.get("sync_info") or {}).get("on_update") or []:
                    incers.setdefault(u["id"], set()).add(
                        (ins.get("engine"), ins.get("opcode"))
                    )
    n_ev = 0
    for fn in d["functions"]:
        for bb in fn.get("blocks") or []:
            out = []
            for ins in bb["instructions"]:
                si = ins.get("sync_info")
                waits = (si or {}).get("on_wait") or []
                opcode = ins.get("opcode")
                if (
                    si
                    and len(waits) > max_waits
                    and opcode != "EventSemaphore"
                ):
                    eng = ins.get("engine")
                    kept = []
                    for w in waits:
                        nm = w.get("ant_name", "")
                        srcs = incers.get(w.get("id"), set())
                        if (
                            _ENGINE_SEM.match(nm)
                            and nm.startswith(str(eng) + "_")
                            and srcs
                            and all(
                                e == eng and op != "DMACopy" for e, op in srcs
                            )
                        ):
                            # Same-engine program-order wait: every inc comes
                            # from an earlier instruction on this in-order
                            # engine, so it holds by the time this issues.
                            continue
                        kept.append(w)
                    for w in kept[max_waits:]:
                        n_ev += 1
                        out.append(
                            {
                                "debug": ins.get("debug", 0),
                                "engine": eng,
                                "ins": [],
                                "outs": [],
                                "name": f"evw-{n_ev}",
                                "opcode": "EventSemaphore",
                                "sync_info": {"on_update": [], "on_wait": [w]},
                            }
                        )
                    si["on_wait"] = kept[:max_waits]
                out.append(ins)
            bb["instructions"] = out
    return json.dumps(d).encode()


class _WaitLegalBass(bass.Bass):
    def to_json_bytes(self) -> bytes:
        return _legalize_bir_waits(super().to_json_bytes())


def _bcast_c(ap: bass.AP, ncols: int = C) -> bass.AP:
    """Broadcast a [P, n] AP over a trailing axis of ncols via stride 0."""
    return bass.AP(
        tensor=ap.tensor,
        offset=ap.offset,
        ap=[ap.ap[0], ap.ap[1], [0, ncols]],
    )


def build_nc(chain: int = 1, loop_iters: int = 0) -> bass.Bass:
    """Build the LRN kernel.

    chain > 1 repeats the identical full pass (same x -> same y) inline.
    loop_iters > 0 additionally wraps the passes in a tc.For_i hardware loop.
    """
    import contextlib

    assert W % N_IN_CHUNK == 0 and W % N_CCHUNK == 0
    win_ch = W // N_IN_CHUNK
    wc_ch = W // N_CCHUNK

    nc = _WaitLegalBass(trn_type="TRN2")
    x = nc.dram_tensor("x", [ROWS, W, C], IN_DTYPE, kind="ExternalInput")
    y = nc.dram_tensor("y", [ROWS, W, C], OUT_DTYPE, kind="ExternalOutput")

    with tile.TileContext(nc) as tc:
        with (
            tc.tile_pool(name="xpool", bufs=XPOOL_BUFS) as xpool,
            tc.tile_pool(name="x2pool", bufs=X2POOL_BUFS) as x2pool,
            tc.tile_pool(name="opool", bufs=OPOOL_BUFS) as opool,
            tc.tile_pool(name="spool", bufs=SPOOL_BUFS) as spool,
            tc.tile_pool(name="wpool", bufs=WPOOL_BUFS) as wpool,
            tc.tile_pool(name="rpool", bufs=RPOOL_BUFS) as rpool,
        ):

            def emit_pass():
                for it in range(NTILES):
                    r0 = it * P
                    # Input DMA on the SP HWDGE queue. Each SBUF tile spans
                    # win_ch columns; the first tile fills each via finer
                    # sub-DMAs so compute starts sooner.
                    x_in = []
                    n_sub = (
                        max(1, FIRST_TILE_IN_CHUNKS // N_IN_CHUNK)
                        if it == 0
                        else 1
                    )
                    sub = win_ch // n_sub
                    for jc in range(N_IN_CHUNK):
                        w0 = jc * win_ch
                        xc = xpool.tile([P, win_ch, C], IN_DTYPE)
                        for js in range(n_sub):
                            nc.sync.dma_start(
                                out=xc[:, js * sub : (js + 1) * sub, :],
                                in_=x[
                                    r0 : r0 + P,
                                    w0 + js * sub : w0 + (js + 1) * sub,
                                    :,
                                ],
                            )
                        x_in.append(xc)

                    def xslice(w0, wlen):
                        """View [P, wlen, C] of the input at w-offset w0."""
                        jc = w0 // win_ch
                        assert (w0 + wlen - 1) // win_ch == jc
                        off = w0 - jc * win_ch
                        return x_in[jc][:, off : off + wlen, :]

                    if DIAG_MODE == "dma_only":
                        for jc in range(N_IN_CHUNK):
                            w0 = jc * win_ch
                            dma_eng = nc.scalar if OUT_DMA_ON_ACT else nc.sync
                            dma_eng.dma_start(
                                out=y[r0 : r0 + P, w0 : w0 + win_ch, :],
                                in_=x_in[jc],
                            )
                        continue

                    if DIAG_MODE == "square_only":
                        for jc in range(N_CCHUNK):
                            w0 = jc * wc_ch
                            x2 = x2pool.tile([P, wc_ch, C], X2_DTYPE)
                            nc.scalar.square(x2, xslice(w0, wc_ch))
                            # Token DMA so the squares aren't dead code.
                            nc.scalar.dma_start(
                                out=y[r0 : r0 + P, jc : jc + 1, :],
                                in_=x2[:, 0:1, 0:32].bitcast(OUT_DTYPE),
                            )
                        continue

                    # s_pad holds the C-sums, 5-wide zero border each side.
                    s_pad = spool.tile([P, WPAD], _F32)
                    nc.gpsimd.memset(s_pad[:, 0:RADIUS], 0.0)
                    nc.gpsimd.memset(s_pad[:, W + RADIUS : WPAD], 0.0)

                    for jc in range(N_CCHUNK):
                        w0 = jc * wc_ch
                        x2 = x2pool.tile([P, wc_ch, C], X2_DTYPE)
                        # Square on ACT; grouped C-sum on DVE.
                        nc.scalar.square(x2, xslice(w0, wc_ch))
                        nc.vector.reduce_sum(
                            out=s_pad[:, RADIUS + w0 : RADIUS + w0 + wc_ch],
                            in_=x2,
                            axis=mybir.AxisListType.X,
                        )

                    if DIAG_MODE == "reduce_only":
                        # Token DMA so the reduces aren't dead code.
                        nc.scalar.dma_start(
                            out=y[r0 : r0 + P, 0:1, :],
                            in_=s_pad[:, 0:32].bitcast(OUT_DTYPE),
                        )
                        continue

                    # Sliding-window sum of width 11 via log-shift adds,
                    # then denom = sqrt(alpha*win + bias); rstd = 1/denom.
                    weng = nc.gpsimd if WINDOW_ON_POOL else nc.vector
                    rstd = wpool.tile([P, W], _F32)
                    hw0 = W // N_WIN_SPLIT
                    halves = [(i * hw0, hw0) for i in range(N_WIN_SPLIT)]
                    prio_cm = (
                        tc.high_priority(offset=WINDOW_PRIO_OFFSET)
                        if WINDOW_PRIO_OFFSET
                        else contextlib.nullcontext()
                    )
                    engs = MUL_ENGINES[it % len(MUL_ENGINES)]
                    rb2 = {}
                    with prio_cm:
                        for h0, hw in halves:
                            # win[w] = sum_{d=0..10} s_pad[w+d], w in
                            # [h0,h0+hw): needs s_pad[h0 : h0+hw+10].
                            sl = s_pad[:, h0 : h0 + hw + 10]
                            a = wpool.tile([P, hw + 9], _F32, tag="wa")
                            weng.tensor_add(
                                a, sl[:, 0 : hw + 9], sl[:, 1 : hw + 10]
                            )
                            b = wpool.tile([P, hw + 7], _F32, tag="wb")
                            weng.tensor_add(
                                b, a[:, 0 : hw + 7], a[:, 2 : hw + 9]
                            )
                            c = wpool.tile([P, hw + 3], _F32, tag="wc")
                            weng.tensor_add(
                                c, b[:, 0 : hw + 3], b[:, 4 : hw + 7]
                            )
                            d = wpool.tile([P, hw], _F32, tag="wd")
                            weng.tensor_add(d, c[:, 0:hw], a[:, 8 : hw + 8])
                            win = wpool.tile([P, hw], _F32, tag="wwin")
                            weng.tensor_add(win, d, sl[:, 10 : hw + 10])
                            denom = wpool.tile([P, hw], _F32, tag="wden")
                            nc.scalar.activation(
                                out=denom,
                                in_=win,
                                func=mybir.ActivationFunctionType.Sqrt,
                                bias=BIAS,
                                scale=ALPHA,
                            )
                            nc.vector.reciprocal(
                                out=rstd[:, h0 : h0 + hw], in_=denom
                            )
                            if "u" in "".join(engs):
                                # Paired-bf16 rstd: rb2[p,w] = {bf16(r),
                                # bf16(r)} so a u32 broadcast-copy moves two
                                # bf16 per ACT element.
                                t2 = wpool.tile([P, hw, 2], OUT_DTYPE, tag="wrb2")
                                nc.scalar.activation(
                                    out=t2,
                                    in_=_bcast_c(rstd[:, h0 : h0 + hw], 2),
                                    func=mybir.ActivationFunctionType.Copy,
                                )
                                rb2[h0] = t2

                    # out = x * rstd broadcast over C, chunked.
                    for jc in range(N_CCHUNK):
                        w0 = jc * wc_ch
                        out_c = opool.tile([P, wc_ch, C], OUT_DTYPE)
                        if engs[jc] == "u":
                            hw0_ = W // N_WIN_SPLIT
                            h0 = (w0 // hw0_) * hw0_
                            off = w0 - h0
                            src = rb2[h0][:, off : off + wc_ch, :].bitcast(
                                _F32
                            )
                            rep = rpool.tile(
                                [P, wc_ch, C // 2], _F32, tag="rep"
                            )
                            nc.scalar.activation(
                                out=rep,
                                in_=bass.AP(
                                    tensor=src.tensor,
                                    offset=src.offset,
                                    ap=[src.ap[0], src.ap[1], [0, C // 2]],
                                ),
                                func=mybir.ActivationFunctionType.Copy,
                            )
                            nc.vector.tensor_mul(
                                out_c,
                                xslice(w0, wc_ch),
                                rep[:, :, :].bitcast(OUT_DTYPE),
                            )
                        elif engs[jc] in ("a", "b"):
                            # Materialize the broadcast rstd as bf16 so the
                            # DVE mul qualifies for 2x_1P (all operands
                            # 16-bit, unit stride).
                            rep = rpool.tile([P, wc_ch, C], OUT_DTYPE)
                            bc = _bcast_c(rstd[:, w0 : w0 + wc_ch])
                            if engs[jc] == "a":
                                nc.scalar.activation(
                                    out=rep,
                                    in_=bc,
                                    func=mybir.ActivationFunctionType.Copy,
                                )
                            else:
                                nc.gpsimd.tensor_scalar_mul(rep, bc, 1.0)
                            nc.vector.tensor_mul(
                                out_c, xslice(w0, wc_ch), rep
                            )
                        elif engs[jc] == "h":
                            # Channel-split: DVE low cols, Pool high cols.
                            cd = C - POOL_COLS
                            xs = xslice(w0, wc_ch)
                            rs = rstd[:, w0 : w0 + wc_ch]

                            def _bc(rap, ncols):
                                return bass.AP(
                                    tensor=rap.tensor,
                                    offset=rap.offset,
                                    ap=[rap.ap[0], rap.ap[1], [0, ncols]],
                                )

                            nc.vector.tensor_mul(
                                out_c[:, :, 0:cd], xs[:, :, 0:cd], _bc(rs, cd)
                            )
                            nc.gpsimd.tensor_mul(
                                out_c[:, :, cd:C],
                                xs[:, :, cd:C],
                                _bc(rs, C - cd),
                            )
                        else:
                            on_pool = engs[jc] == "p"
                            eng = nc.gpsimd if on_pool else nc.vector
                            nsp = POOL_MUL_SPLIT if on_pool else 1
                            msub = wc_ch // nsp
                            for js in range(nsp):
                                o0 = js * msub
                                eng.tensor_mul(
                                    out_c[:, o0 : o0 + msub, :],
                                    xslice(w0 + o0, msub),
                                    _bcast_c(
                                        rstd[:, w0 + o0 : w0 + o0 + msub]
                                    ),
                                )
                        dma_eng = nc.scalar if OUT_DMA_ON_ACT else nc.sync
                        dma_eng.dma_start(
                            out=y[r0 : r0 + P, w0 : w0 + wc_ch, :], in_=out_c
                        )

            loop_cm = (
                tc.For_i(0, loop_iters)
                if loop_iters > 0
                else contextlib.nullcontext()
            )
            with loop_cm:
                for _rep in range(chain):
                    emit_pass()

    return nc
